# revision 9
# baseline (speedup 1.0000x reference)
"""CRF NLL loss kernel for Trainium2 (8 NeuronCores, batch-sharded).

Strategy (v2: time-segmented, rank-1 stitched)
----------------------------------------------
Data-parallel over batch: each core handles BC=64 sequences.  The T=512
forward recurrence (exp domain, labels on partitions, batch on free dim)

    w_t = e_t * (E'^T w_{t-1}),   E' = exp(transitions - C0)

is split into S=16 time segments of LEN=32 steps.  Products of these
strictly positive matrices contract to rank-1 within a few steps, so each
interior segment operator P_s is summarized by its action on probes:
  fwd chains  a_s = P_s r            (seg 0 uses the true init)
  bwd chains  X_s = q^T P_s          merged with EOS-capture injection:
              columns whose sequence ends (t* = len-1) inside seg s carry
              init/injection texp = exp(trans[:,EOS]) at step t* instead
              of the probe; host selects per column.
All 30 chains advance in lockstep rounds r=0..31; per round the DVE does
two joint [128, 960] PSUM->SBUF multiply ops (one per direction), the PE
does 30 matmuls (shared stationaries E', E'^T) plus rank-1 injection
accumulates (stationary texp row).  Features stream in a host-permuted
"need order" (round-major) so DMA/ACT-exp stay ahead of compute with an
8-slot ring; every step's e_t appears twice (once fwd, once bwd).

Host post-processing stitches segments per column with rank-1 junctions
(all dots in float64) and restores the t*.C0 shift; gold path scores are
index-gathered on host and summed on device, as before.
"""

import numpy as np

B, T, L = 512, 512, 128
NCORES = 8
BC = B // NCORES            # 64 sequences per core
PAD, BOS, EOS = 0, 1, 2
C0 = 5.0
S = 16                      # time segments
LEN = 32                    # steps per segment (seg 0: steps 1..32, ...,
                            # seg 15: steps 481..511 -> 31 rounds)
NF = 15                     # fwd chains  (segments 0..14)
NB = 15                     # bwd chains  (segments 1..15)
NSL = NF + NB               # feature slices per round group
NGRP = 33                   # stream groups: [t0] + 32 rounds
RING = 8                    # DMA/exp ring depth (groups)
TMIN = 255                  # smallest possible t* (len >= 256)

F32 = np.float32

_compiled = None


def _t_end(s):
    return 32 * (s + 1) if s <= 14 else 511


def _t_bwd(s, r):
    return 32 * (s + 1) - r if s <= 14 else 511 - r


def _build():
    import concourse.bass as bass
    import concourse.bacc as bacc
    import concourse.mybir as mybir
    import concourse.tile as tile

    f32 = mybir.dt.float32
    bf16 = mybir.dt.bfloat16
    nc = bacc.Bacc("TRN2", target_bir_lowering=False, debug=False)

    featp = nc.dram_tensor("featp", [L, (1 + 32 * NSL) * BC], bf16,
                           kind="ExternalInput")
    trans = nc.dram_tensor("trans", [L, L], f32, kind="ExternalInput")
    trans_t = nc.dram_tensor("trans_t", [L, L], f32, kind="ExternalInput")
    qmask_d = nc.dram_tensor("qmask", [1, NB * BC], bf16, kind="ExternalInput")
    indend_d = nc.dram_tensor("indend", [1, NB * BC], bf16, kind="ExternalInput")
    indstk_d = nc.dram_tensor("indstk", [1, 257 * BC], bf16, kind="ExternalInput")
    emis_v = nc.dram_tensor("emis_v", [BC, T], f32, kind="ExternalInput")
    emis_w = nc.dram_tensor("emis_w", [BC, T], f32, kind="ExternalInput")
    trans_v = nc.dram_tensor("trans_v", [BC, T + 1], f32, kind="ExternalInput")
    trans_w = nc.dram_tensor("trans_w", [BC, T + 1], f32, kind="ExternalInput")

    aout = nc.dram_tensor("aout", [L, NF * BC], f32, kind="ExternalOutput")
    xout = nc.dram_tensor("xout", [L, NB * BC], f32, kind="ExternalOutput")
    gold_o = nc.dram_tensor("gold", [BC, 1], f32, kind="ExternalOutput")

    AX = mybir.AxisListType.X
    MUL = mybir.AluOpType.mult
    ADD = mybir.AluOpType.add
    EXP = mybir.ActivationFunctionType.Exp

    GW = NSL * BC               # 1920: cols per full ring group
    FW = NF * BC                # 960

    with tile.TileContext(nc) as tc:
        with (
            tc.tile_pool(name="st", bufs=1) as st,
            tc.tile_pool(name="raws", bufs=RING) as rawp,
            tc.tile_pool(name="efs", bufs=RING) as efp,
            tc.tile_pool(name="pfp", bufs=1, space="PSUM") as pfp,
            tc.tile_pool(name="pbp", bufs=1, space="PSUM") as pbp,
        ):
            # ---- one-time setup ----
            tr_sb = st.tile([L, L], f32)
            nc.sync.dma_start(tr_sb[:], trans[:])
            trt_sb = st.tile([L, L], f32)
            nc.sync.dma_start(trt_sb[:], trans_t[:])
            nc0 = st.tile([L, 1], f32)          # bias tiles
            nc.vector.memset(nc0[:], -C0)
            zb = st.tile([L, 1], f32)
            nc.vector.memset(zb[:], 0.0)
            zb1 = st.tile([1, 1], f32)
            nc.vector.memset(zb1[:], 0.0)
            Ep = st.tile([L, L], bf16)          # E' = exp(trans - C0)
            nc.scalar.activation(Ep[:], tr_sb[:], EXP, bias=nc0[:], scale=1.0)
            EpT = st.tile([L, L], bf16)         # E'^T
            nc.scalar.activation(EpT[:], trt_sb[:], EXP, bias=nc0[:],
                                 scale=1.0)
            texp_f = st.tile([1, L], f32)
            nc.sync.dma_start(texp_f[:],
                              trans[:, EOS:EOS + 1].rearrange("a b -> b a"))
            texp_row = st.tile([1, L], bf16)    # exp(trans[:, EOS]) as a row
            nc.scalar.activation(texp_row[:], texp_f[:], EXP,
                                 bias=zb1[:], scale=1.0)
            ebos = st.tile([L, 1], f32)         # exp(trans[BOS, :]) column
            nc.sync.dma_start(ebos[:],
                              trans[BOS:BOS + 1, :].rearrange("a b -> b a"))
            nc.scalar.activation(ebos[:], ebos[:], EXP, bias=zb[:], scale=1.0)
            ones_row = st.tile([1, L], bf16)
            nc.vector.memset(ones_row[:], 1.0)
            qmask_bf = st.tile([1, NB * BC], bf16)
            nc.sync.dma_start(qmask_bf[:], qmask_d[:])
            indend_bf = st.tile([1, NB * BC], bf16)
            nc.sync.dma_start(indend_bf[:], indend_d[:])
            indstk_bf = st.tile([1, 257 * BC], bf16)
            nc.sync.dma_start(indstk_bf[:], indstk_d[:])

            wring = st.tile([L, FW], bf16)      # fwd states, chain-major
            dring = st.tile([L, FW], bf16)      # bwd delta tiles
            pf = pfp.tile([L, FW], f32, space="PSUM")
            pb = pbp.tile([L, FW], f32, space="PSUM")

            # ---- feature stream: group 0 = t0 slice, groups 1+g = round g
            raw_tiles, ef_tiles = [], []

            def pump(g):
                # DMA + exp one stream group into the rings
                if g == 0:
                    cols = BC
                    base = 0
                else:
                    cols = GW
                    base = BC + (g - 1) * GW
                rt = rawp.tile([L, GW], bf16, tag="raw", name=f"raw{g}")
                nc.sync.dma_start(rt[:, 0:cols], featp[:, base:base + cols])
                et = efp.tile([L, GW], bf16, tag="ef", name=f"ef{g}")
                nc.scalar.activation(et[:, 0:cols], rt[:, 0:cols], EXP,
                                     bias=zb[:], scale=1.0)
                raw_tiles.append(rt)
                ef_tiles.append(et)

            for g in range(RING):
                pump(g)

            # ---- init states ----
            # w0 = ebos * e_0 for the true chain; probes = 1
            nc.vector.tensor_scalar(out=wring[:, 0:BC],
                                    in0=ef_tiles[0][:, 0:BC],
                                    scalar1=ebos[:, 0:1], scalar2=None,
                                    op0=MUL)
            nc.vector.memset(wring[:, BC:FW], 1.0)

            # bwd inits into pb: q-mask + texp (x) ind_end per chain
            for s in range(1, S):
                sl = pb[:, (s - 1) * BC:s * BC]
                nc.tensor.matmul(sl, lhsT=ones_row[:],
                                 rhs=qmask_bf[0:1, (s - 1) * BC:s * BC],
                                 start=True, stop=False, skip_group_check=True)
                nc.tensor.matmul(sl, lhsT=texp_row[:],
                                 rhs=indend_bf[0:1, (s - 1) * BC:s * BC],
                                 start=False, stop=True, skip_group_check=True)

            # ---- main rounds ----
            # per round r: bwd_DVE(r) [delta = ef_r * pb(r-1)], fwd MMs(r),
            # bwd MMs(r) [consume delta(r), + rank-1 injection], fwd_DVE(r).
            # DVE alternates bwd/fwd joints while PE alternates fwd/bwd MMs.
            for r in range(LEN):
                et = ef_tiles[r + 1]
                nbw = NB if r < LEN - 1 else NB - 1   # chain 15 idle at r=31

                # bwd joint DVE: delta = ef * pb  (pb = last round's outputs)
                nc.vector.tensor_tensor(out=dring[:, 0:nbw * BC],
                                        in0=pb[:, 0:nbw * BC],
                                        in1=et[:, FW:FW + nbw * BC], op=MUL)

                # fwd matmuls (stationary Ep), all 15 chains
                for k in range(NF):
                    nc.tensor.matmul(pf[:, k * BC:(k + 1) * BC], lhsT=Ep[:],
                                     rhs=wring[:, k * BC:(k + 1) * BC],
                                     start=True, stop=True)

                # bwd main matmuls (stationary EpT), each immediately
                # followed by its rank-1 injection accumulate: start=True
                # clears has_written for the WHOLE bank, so an accumulate
                # must not be separated from its main by another chain's
                # start=True in the same bank.
                for k in range(nbw):
                    s = k + 1
                    tn = _t_bwd(s, r) - 1
                    has_inj = TMIN <= tn <= 511 and tn >= 32 * s + 1
                    nc.tensor.matmul(pb[:, k * BC:(k + 1) * BC], lhsT=EpT[:],
                                     rhs=dring[:, k * BC:(k + 1) * BC],
                                     start=True, stop=not has_inj,
                                     skip_group_check=True)
                    if has_inj:
                        off = (tn - TMIN) * BC
                        nc.tensor.matmul(
                            pb[:, k * BC:(k + 1) * BC], lhsT=texp_row[:],
                            rhs=indstk_bf[0:1, off:off + BC],
                            start=False, stop=True, skip_group_check=True)

                # fwd joint DVE: w = ef * pf
                nc.vector.tensor_tensor(out=wring[:], in0=pf[:],
                                        in1=et[:, 0:FW], op=MUL)

                # keep the feature ring pumped
                g = r + RING
                if g < NGRP:
                    pump(g)

            # ---- outputs ----
            ast = st.tile([L, FW], f32)
            nc.vector.tensor_copy(ast[:], wring[:])
            nc.sync.dma_start(aout[:], ast[:])
            xst = st.tile([L, FW], f32)
            nc.vector.tensor_copy(xst[:], pb[:])
            nc.sync.dma_start(xout[:], xst[:])

            # ---- gold score masked sums ----
            ev_sb = st.tile([BC, T], f32)
            nc.sync.dma_start(ev_sb[:], emis_v[:])
            ew_sb = st.tile([BC, T], f32)
            nc.sync.dma_start(ew_sb[:], emis_w[:])
            nc.vector.tensor_tensor(out=ev_sb[:], in0=ev_sb[:], in1=ew_sb[:],
                                    op=MUL)
            g1 = st.tile([BC, 1], f32)
            nc.vector.reduce_sum(g1[:], ev_sb[:], axis=AX)
            tv_sb = st.tile([BC, T + 1], f32)
            nc.sync.dma_start(tv_sb[:], trans_v[:])
            tw_sb = st.tile([BC, T + 1], f32)
            nc.sync.dma_start(tw_sb[:], trans_w[:])
            nc.vector.tensor_tensor(out=tv_sb[:], in0=tv_sb[:], in1=tw_sb[:],
                                    op=MUL)
            g2 = st.tile([BC, 1], f32)
            nc.vector.reduce_sum(g2[:], tv_sb[:], axis=AX)
            nc.vector.tensor_tensor(out=g1[:], in0=g1[:], in1=g2[:], op=ADD)
            nc.sync.dma_start(gold_o[:], g1[:])

    nc.compile()
    return nc


def _get_compiled():
    global _compiled
    if _compiled is None:
        _compiled = _build()
    return _compiled


def _bf16(a):
    """fp32 -> bf16 numpy array (ml_dtypes), round-to-nearest-even."""
    import ml_dtypes

    return a.astype(ml_dtypes.bfloat16)


def _stream_perm():
    """Position -> time for the permuted feature stream."""
    pi = np.zeros(1 + 32 * NSL, dtype=np.int64)
    pi[0] = 0
    for r in range(32):
        for k in range(NSL):
            if k < NF:
                t = 32 * k + r + 1
            else:
                t = _t_bwd(k - NF + 1, r)
            pi[1 + r * NSL + k] = t
    return pi


_PI = _stream_perm()


def _prep_core(feat, tags, maskf, trans_np):
    """Host-side marshalling for one core's shard."""
    lens = maskf.sum(axis=1).astype(np.int64)
    tstar = lens - 1                                   # in [255, 511]

    fT = np.ascontiguousarray(feat.transpose(2, 1, 0))  # [L, T, BC]
    featp = fT[:, _PI, :].reshape(L, -1)               # [L, 961*BC]
    featp = _bf16(np.ascontiguousarray(featp))

    qmask = np.zeros((1, NB * BC), dtype=F32)
    indend = np.zeros((1, NB * BC), dtype=F32)
    for s in range(1, S):
        te = _t_end(s)
        qmask[0, (s - 1) * BC:s * BC] = (tstar > te)
        indend[0, (s - 1) * BC:s * BC] = (tstar == te)

    indstk = np.zeros((1, 257 * BC), dtype=F32)
    for t in range(TMIN, 512):
        indstk[0, (t - TMIN) * BC:(t - TMIN + 1) * BC] = (tstar == t)

    emis_v = np.take_along_axis(feat, tags[..., None], axis=-1)[..., 0]
    emis_w = maskf.copy()
    emis_w[:, 0] = 1.0
    trans_v = np.empty((BC, T + 1), dtype=F32)
    trans_v[:, : T - 1] = trans_np[tags[:, :-1], tags[:, 1:]]
    trans_v[:, T - 1] = trans_np[BOS, tags[:, 0]]
    last_lab = tags[np.arange(BC), tstar]
    trans_v[:, T] = trans_np[last_lab, EOS]
    trans_w = np.empty((BC, T + 1), dtype=F32)
    trans_w[:, : T - 1] = maskf[:, 1:]
    trans_w[:, T - 1] = 1.0
    trans_w[:, T] = 1.0

    in_map = {
        "featp": featp,
        "trans": np.ascontiguousarray(trans_np),
        "trans_t": np.ascontiguousarray(trans_np.T),
        "qmask": _bf16(qmask),
        "indend": _bf16(indend),
        "indstk": _bf16(indstk),
        "emis_v": np.ascontiguousarray(emis_v.astype(F32)),
        "emis_w": np.ascontiguousarray(emis_w),
        "trans_v": trans_v,
        "trans_w": trans_w,
    }
    return in_map, tstar


def _prep_all(inputs):
    feats = np.asarray(inputs["features"], dtype=F32)
    tags = np.asarray(inputs["tag_seqs"])
    maskf = np.asarray(inputs["mask"]).astype(F32)
    tr = np.asarray(inputs["transitions"], dtype=F32)
    in_maps = []
    for c in range(NCORES):
        sl = slice(c * BC, (c + 1) * BC)
        m, _ = _prep_core(feats[sl], tags[sl], maskf[sl], tr)
        in_maps.append(m)
    return in_maps


def _stitch_core(out, tstar):
    """Host rank-1 stitching for one core -> per-seq (gold - logZ)."""
    a = np.asarray(out["aout"], dtype=np.float64).reshape(L, NF, BC)
    X = np.asarray(out["xout"], dtype=np.float64).reshape(L, NB, BC)
    gold = np.asarray(out["gold"], dtype=np.float64)[:, 0]

    # dots: m1[s] = log(X_{s+1} . a_s)  s=1..14 ; mfin = log(X_1 . a_0)
    # msum[s] = log(sum a_s)
    with np.errstate(divide="ignore"):
        m1 = np.log(np.einsum("lsb,lsb->sb", X[:, 1:, :], a[:, 1:, :]))
        msum = np.log(a[:, 1:, :].sum(axis=0))         # [14, BC] s=1..14
        mfin = np.log(np.einsum("lb,lb->b", X[:, 0, :], a[:, 0, :]))

    sstar = (tstar - 1) // 32                          # in [7, 15]
    # logZdev = sum_{s=1}^{sstar-1} (m1[s] - msum[s]) + mfin
    # m1 index: s -> m1[s-1]
    cum = np.concatenate([np.zeros((1, BC)),
                          np.cumsum(m1 - msum, axis=0)], axis=0)  # [15, BC]
    logZdev = cum[sstar - 1, np.arange(BC)] + mfin
    logZ = logZdev + tstar * C0
    return gold - logZ


def kernel(features, tag_seqs, mask, transitions):
    from concourse import bass_utils

    feats = np.asarray(features, dtype=F32)
    tags = np.asarray(tag_seqs)
    maskf = np.asarray(mask).astype(F32)
    trans_np = np.asarray(transitions, dtype=F32)

    nc = _get_compiled()

    in_maps, tstars = [], []
    for c in range(NCORES):
        sl = slice(c * BC, (c + 1) * BC)
        m, ts = _prep_core(feats[sl], tags[sl], maskf[sl], trans_np)
        in_maps.append(m)
        tstars.append(ts)

    res = bass_utils.run_bass_kernel_spmd(nc, in_maps,
                                          core_ids=list(range(NCORES)))

    per_seq = [_stitch_core(res.results[c], tstars[c]) for c in range(NCORES)]
    loss = -np.mean(np.concatenate(per_seq))
    return np.float32(loss)


# revision 10
# speedup vs baseline: 1.0034x; 1.0034x over previous
"""CRF NLL loss kernel for Trainium2 (8 NeuronCores, batch-sharded).

Strategy (v2: time-segmented, rank-1 stitched)
----------------------------------------------
Data-parallel over batch: each core handles BC=64 sequences.  The T=512
forward recurrence (exp domain, labels on partitions, batch on free dim)

    w_t = e_t * (E'^T w_{t-1}),   E' = exp(transitions - C0)

is split into S=16 time segments of LEN=32 steps.  Products of these
strictly positive matrices contract to rank-1 within a few steps, so each
interior segment operator P_s is summarized by its action on probes:
  fwd chains  a_s = P_s r            (seg 0 uses the true init)
  bwd chains  X_s = q^T P_s          merged with EOS-capture injection:
              columns whose sequence ends (t* = len-1) inside seg s carry
              init/injection texp = exp(trans[:,EOS]) at step t* instead
              of the probe; host selects per column.
All 30 chains advance in lockstep rounds r=0..31; per round the DVE does
two joint [128, 960] PSUM->SBUF multiply ops (one per direction), the PE
does 30 matmuls (shared stationaries E', E'^T) plus rank-1 injection
accumulates (stationary texp row).  Features stream in a host-permuted
"need order" (round-major) so DMA/ACT-exp stay ahead of compute with an
8-slot ring; every step's e_t appears twice (once fwd, once bwd).

Host post-processing stitches segments per column with rank-1 junctions
(all dots in float64) and restores the t*.C0 shift; gold path scores are
index-gathered on host and summed on device, as before.
"""

import numpy as np

B, T, L = 512, 512, 128
NCORES = 8
BC = B // NCORES            # 64 sequences per core
PAD, BOS, EOS = 0, 1, 2
C0 = 5.0
S = 16                      # time segments
LEN = 32                    # steps per segment (seg 0: steps 1..32, ...,
                            # seg 15: steps 481..511 -> 31 rounds)
NF = 15                     # fwd chains  (segments 0..14)
NB = 15                     # bwd chains  (segments 1..15)
NSL = NF + NB               # feature slices per round group
NGRP = 33                   # stream groups: [t0] + 32 rounds
RING = 8                    # DMA/exp ring depth (groups)
TMIN = 255                  # smallest possible t* (len >= 256)

F32 = np.float32

_compiled = None


def _t_end(s):
    return 32 * (s + 1) if s <= 14 else 511


def _t_bwd(s, r):
    return 32 * (s + 1) - r if s <= 14 else 511 - r


def _build():
    import concourse.bass as bass
    import concourse.bacc as bacc
    import concourse.mybir as mybir
    import concourse.tile as tile

    f32 = mybir.dt.float32
    bf16 = mybir.dt.bfloat16
    i32 = mybir.dt.int32
    nc = bacc.Bacc("TRN2", target_bir_lowering=False, debug=False)

    featp = nc.dram_tensor("featp", [L, (1 + 32 * NSL) * BC], bf16,
                           kind="ExternalInput")
    trans = nc.dram_tensor("trans", [L, L], f32, kind="ExternalInput")
    trans_t = nc.dram_tensor("trans_t", [L, L], f32, kind="ExternalInput")
    qmask_d = nc.dram_tensor("qmask", [1, NB * BC], bf16, kind="ExternalInput")
    indend_d = nc.dram_tensor("indend", [1, NB * BC], bf16, kind="ExternalInput")
    indstk_d = nc.dram_tensor("indstk", [1, 257 * BC], bf16, kind="ExternalInput")
    emis_v = nc.dram_tensor("emis_v", [BC, T], f32, kind="ExternalInput")
    emis_w = nc.dram_tensor("emis_w", [BC, T], f32, kind="ExternalInput")
    trans_v = nc.dram_tensor("trans_v", [BC, T + 1], f32, kind="ExternalInput")
    trans_w = nc.dram_tensor("trans_w", [BC, T + 1], f32, kind="ExternalInput")

    aout = nc.dram_tensor("aout", [L, NF * BC], f32, kind="ExternalOutput")
    xout = nc.dram_tensor("xout", [L, NB * BC], f32, kind="ExternalOutput")
    gold_o = nc.dram_tensor("gold", [BC, 1], f32, kind="ExternalOutput")

    AX = mybir.AxisListType.X
    MUL = mybir.AluOpType.mult
    ADD = mybir.AluOpType.add
    EXP = mybir.ActivationFunctionType.Exp

    GW = NSL * BC               # 1920: cols per full ring group
    FW = NF * BC                # 960

    with tile.TileContext(nc) as tc:
        with (
            tc.tile_pool(name="st", bufs=1) as st,
            tc.tile_pool(name="raws", bufs=RING) as rawp,
            tc.tile_pool(name="efs", bufs=RING) as efp,
            tc.tile_pool(name="pfp", bufs=1, space="PSUM") as pfp,
            tc.tile_pool(name="pbp", bufs=1, space="PSUM") as pbp,
        ):
            # ---- one-time setup ----
            tr_sb = st.tile([L, L], f32)
            nc.sync.dma_start(tr_sb[:], trans[:])
            trt_sb = st.tile([L, L], f32)
            nc.sync.dma_start(trt_sb[:], trans_t[:])
            nc0 = st.tile([L, 1], f32)          # bias tiles
            nc.vector.memset(nc0[:], -C0)
            zb = st.tile([L, 1], f32)
            nc.vector.memset(zb[:], 0.0)
            zb1 = st.tile([1, 1], f32)
            nc.vector.memset(zb1[:], 0.0)
            Ep = st.tile([L, L], bf16)          # E' = exp(trans - C0)
            nc.scalar.activation(Ep[:], tr_sb[:], EXP, bias=nc0[:], scale=1.0)
            EpT = st.tile([L, L], bf16)         # E'^T
            nc.scalar.activation(EpT[:], trt_sb[:], EXP, bias=nc0[:],
                                 scale=1.0)
            texp_f = st.tile([1, L], f32)
            nc.sync.dma_start(texp_f[:],
                              trans[:, EOS:EOS + 1].rearrange("a b -> b a"))
            texp_row = st.tile([1, L], bf16)    # exp(trans[:, EOS]) as a row
            nc.scalar.activation(texp_row[:], texp_f[:], EXP,
                                 bias=zb1[:], scale=1.0)
            ebos = st.tile([L, 1], f32)         # exp(trans[BOS, :]) column
            nc.sync.dma_start(ebos[:],
                              trans[BOS:BOS + 1, :].rearrange("a b -> b a"))
            nc.scalar.activation(ebos[:], ebos[:], EXP, bias=zb[:], scale=1.0)
            ones_row = st.tile([1, L], bf16)
            nc.vector.memset(ones_row[:], 1.0)
            qmask_bf = st.tile([1, NB * BC], bf16)
            nc.sync.dma_start(qmask_bf[:], qmask_d[:])
            indend_bf = st.tile([1, NB * BC], bf16)
            nc.sync.dma_start(indend_bf[:], indend_d[:])
            indstk_bf = st.tile([1, 257 * BC], bf16)
            nc.sync.dma_start(indstk_bf[:], indstk_d[:])

            wring = st.tile([L, FW], bf16)      # fwd states, chain-major
            dring = st.tile([L, FW], bf16)      # bwd delta tiles
            pf = pfp.tile([L, FW], f32, space="PSUM")
            pb = pbp.tile([L, FW], f32, space="PSUM")

            # ---- feature stream: group 0 = t0 slice, groups 1+g = round g
            raw_tiles, ef_tiles = [], []

            def pump(g):
                # DMA + exp one stream group into the rings
                if g == 0:
                    cols = BC
                    base = 0
                else:
                    cols = GW
                    base = BC + (g - 1) * GW
                rt = rawp.tile([L, GW], bf16, tag="raw", name=f"raw{g}")
                nc.sync.dma_start(rt[:, 0:cols], featp[:, base:base + cols])
                # Schraudolph fast-exp on the (otherwise idle) GpSimd:
                # ef = bitcast_f32(int32(x * 2^23/ln2 + B)) ~ exp(x) +-3%
                et = efp.tile([L, GW], i32, tag="ef", name=f"ef{g}")
                nc.gpsimd.tensor_scalar(out=et[:, 0:cols], in0=rt[:, 0:cols],
                                        scalar1=12102203.0,
                                        scalar2=1064866805.0,
                                        op0=MUL, op1=ADD)
                raw_tiles.append(rt)
                ef_tiles.append(et.bitcast(f32))

            for g in range(RING):
                pump(g)

            # ---- init states ----
            # w0 = ebos * e_0 for the true chain; probes = 1
            nc.vector.tensor_scalar(out=wring[:, 0:BC],
                                    in0=ef_tiles[0][:, 0:BC],
                                    scalar1=ebos[:, 0:1], scalar2=None,
                                    op0=MUL)
            nc.vector.memset(wring[:, BC:FW], 1.0)

            # bwd inits into pb: q-mask + texp (x) ind_end per chain
            for s in range(1, S):
                sl = pb[:, (s - 1) * BC:s * BC]
                nc.tensor.matmul(sl, lhsT=ones_row[:],
                                 rhs=qmask_bf[0:1, (s - 1) * BC:s * BC],
                                 start=True, stop=False, skip_group_check=True)
                nc.tensor.matmul(sl, lhsT=texp_row[:],
                                 rhs=indend_bf[0:1, (s - 1) * BC:s * BC],
                                 start=False, stop=True, skip_group_check=True)

            # ---- main rounds ----
            # per round r: bwd_DVE(r) [delta = ef_r * pb(r-1)], fwd MMs(r),
            # bwd MMs(r) [consume delta(r), + rank-1 injection], fwd_DVE(r).
            # DVE alternates bwd/fwd joints while PE alternates fwd/bwd MMs.
            for r in range(LEN):
                et = ef_tiles[r + 1]
                nbw = NB if r < LEN - 1 else NB - 1   # chain 15 idle at r=31

                # bwd joint DVE: delta = ef * pb  (pb = last round's outputs)
                nc.vector.tensor_tensor(out=dring[:, 0:nbw * BC],
                                        in0=pb[:, 0:nbw * BC],
                                        in1=et[:, FW:FW + nbw * BC], op=MUL)

                # fwd matmuls (stationary Ep), all 15 chains
                for k in range(NF):
                    nc.tensor.matmul(pf[:, k * BC:(k + 1) * BC], lhsT=Ep[:],
                                     rhs=wring[:, k * BC:(k + 1) * BC],
                                     start=True, stop=True)

                # bwd main matmuls (stationary EpT), each immediately
                # followed by its rank-1 injection accumulate: start=True
                # clears has_written for the WHOLE bank, so an accumulate
                # must not be separated from its main by another chain's
                # start=True in the same bank.
                for k in range(nbw):
                    s = k + 1
                    tn = _t_bwd(s, r) - 1
                    has_inj = TMIN <= tn <= 511 and tn >= 32 * s + 1
                    nc.tensor.matmul(pb[:, k * BC:(k + 1) * BC], lhsT=EpT[:],
                                     rhs=dring[:, k * BC:(k + 1) * BC],
                                     start=True, stop=not has_inj,
                                     skip_group_check=True)
                    if has_inj:
                        off = (tn - TMIN) * BC
                        nc.tensor.matmul(
                            pb[:, k * BC:(k + 1) * BC], lhsT=texp_row[:],
                            rhs=indstk_bf[0:1, off:off + BC],
                            start=False, stop=True, skip_group_check=True)

                # fwd joint DVE: w = ef * pf
                nc.vector.tensor_tensor(out=wring[:], in0=pf[:],
                                        in1=et[:, 0:FW], op=MUL)

                # keep the feature ring pumped
                g = r + RING
                if g < NGRP:
                    pump(g)

            # ---- outputs ----
            ast = st.tile([L, FW], f32)
            nc.vector.tensor_copy(ast[:], wring[:])
            nc.sync.dma_start(aout[:], ast[:])
            xst = st.tile([L, FW], f32)
            nc.vector.tensor_copy(xst[:], pb[:])
            nc.sync.dma_start(xout[:], xst[:])

            # ---- gold score masked sums ----
            ev_sb = st.tile([BC, T], f32)
            nc.sync.dma_start(ev_sb[:], emis_v[:])
            ew_sb = st.tile([BC, T], f32)
            nc.sync.dma_start(ew_sb[:], emis_w[:])
            nc.vector.tensor_tensor(out=ev_sb[:], in0=ev_sb[:], in1=ew_sb[:],
                                    op=MUL)
            g1 = st.tile([BC, 1], f32)
            nc.vector.reduce_sum(g1[:], ev_sb[:], axis=AX)
            tv_sb = st.tile([BC, T + 1], f32)
            nc.sync.dma_start(tv_sb[:], trans_v[:])
            tw_sb = st.tile([BC, T + 1], f32)
            nc.sync.dma_start(tw_sb[:], trans_w[:])
            nc.vector.tensor_tensor(out=tv_sb[:], in0=tv_sb[:], in1=tw_sb[:],
                                    op=MUL)
            g2 = st.tile([BC, 1], f32)
            nc.vector.reduce_sum(g2[:], tv_sb[:], axis=AX)
            nc.vector.tensor_tensor(out=g1[:], in0=g1[:], in1=g2[:], op=ADD)
            nc.sync.dma_start(gold_o[:], g1[:])

    nc.compile()
    return nc


def _get_compiled():
    global _compiled
    if _compiled is None:
        _compiled = _build()
    return _compiled


def _bf16(a):
    """fp32 -> bf16 numpy array (ml_dtypes), round-to-nearest-even."""
    import ml_dtypes

    return a.astype(ml_dtypes.bfloat16)


def _stream_perm():
    """Position -> time for the permuted feature stream."""
    pi = np.zeros(1 + 32 * NSL, dtype=np.int64)
    pi[0] = 0
    for r in range(32):
        for k in range(NSL):
            if k < NF:
                t = 32 * k + r + 1
            else:
                t = _t_bwd(k - NF + 1, r)
            pi[1 + r * NSL + k] = t
    return pi


_PI = _stream_perm()


def _prep_core(feat, tags, maskf, trans_np):
    """Host-side marshalling for one core's shard."""
    lens = maskf.sum(axis=1).astype(np.int64)
    tstar = lens - 1                                   # in [255, 511]

    fT = np.ascontiguousarray(feat.transpose(2, 1, 0))  # [L, T, BC]
    featp = fT[:, _PI, :].reshape(L, -1)               # [L, 961*BC]
    featp = _bf16(np.ascontiguousarray(featp))

    qmask = np.zeros((1, NB * BC), dtype=F32)
    indend = np.zeros((1, NB * BC), dtype=F32)
    for s in range(1, S):
        te = _t_end(s)
        qmask[0, (s - 1) * BC:s * BC] = (tstar > te)
        indend[0, (s - 1) * BC:s * BC] = (tstar == te)

    indstk = np.zeros((1, 257 * BC), dtype=F32)
    for t in range(TMIN, 512):
        indstk[0, (t - TMIN) * BC:(t - TMIN + 1) * BC] = (tstar == t)

    emis_v = np.take_along_axis(feat, tags[..., None], axis=-1)[..., 0]
    emis_w = maskf.copy()
    emis_w[:, 0] = 1.0
    trans_v = np.empty((BC, T + 1), dtype=F32)
    trans_v[:, : T - 1] = trans_np[tags[:, :-1], tags[:, 1:]]
    trans_v[:, T - 1] = trans_np[BOS, tags[:, 0]]
    last_lab = tags[np.arange(BC), tstar]
    trans_v[:, T] = trans_np[last_lab, EOS]
    trans_w = np.empty((BC, T + 1), dtype=F32)
    trans_w[:, : T - 1] = maskf[:, 1:]
    trans_w[:, T - 1] = 1.0
    trans_w[:, T] = 1.0

    in_map = {
        "featp": featp,
        "trans": np.ascontiguousarray(trans_np),
        "trans_t": np.ascontiguousarray(trans_np.T),
        "qmask": _bf16(qmask),
        "indend": _bf16(indend),
        "indstk": _bf16(indstk),
        "emis_v": np.ascontiguousarray(emis_v.astype(F32)),
        "emis_w": np.ascontiguousarray(emis_w),
        "trans_v": trans_v,
        "trans_w": trans_w,
    }
    return in_map, tstar


def _prep_all(inputs):
    feats = np.asarray(inputs["features"], dtype=F32)
    tags = np.asarray(inputs["tag_seqs"])
    maskf = np.asarray(inputs["mask"]).astype(F32)
    tr = np.asarray(inputs["transitions"], dtype=F32)
    in_maps = []
    for c in range(NCORES):
        sl = slice(c * BC, (c + 1) * BC)
        m, _ = _prep_core(feats[sl], tags[sl], maskf[sl], tr)
        in_maps.append(m)
    return in_maps


def _stitch_core(out, tstar):
    """Host rank-1 stitching for one core -> per-seq (gold - logZ)."""
    a = np.asarray(out["aout"], dtype=np.float64).reshape(L, NF, BC)
    X = np.asarray(out["xout"], dtype=np.float64).reshape(L, NB, BC)
    gold = np.asarray(out["gold"], dtype=np.float64)[:, 0]

    # dots: m1[s] = log(X_{s+1} . a_s)  s=1..14 ; mfin = log(X_1 . a_0)
    # msum[s] = log(sum a_s)
    with np.errstate(divide="ignore"):
        m1 = np.log(np.einsum("lsb,lsb->sb", X[:, 1:, :], a[:, 1:, :]))
        msum = np.log(a[:, 1:, :].sum(axis=0))         # [14, BC] s=1..14
        mfin = np.log(np.einsum("lb,lb->b", X[:, 0, :], a[:, 0, :]))

    sstar = (tstar - 1) // 32                          # in [7, 15]
    # logZdev = sum_{s=1}^{sstar-1} (m1[s] - msum[s]) + mfin
    # m1 index: s -> m1[s-1]
    cum = np.concatenate([np.zeros((1, BC)),
                          np.cumsum(m1 - msum, axis=0)], axis=0)  # [15, BC]
    logZdev = cum[sstar - 1, np.arange(BC)] + mfin
    logZ = logZdev + tstar * C0
    return gold - logZ


def kernel(features, tag_seqs, mask, transitions):
    from concourse import bass_utils

    feats = np.asarray(features, dtype=F32)
    tags = np.asarray(tag_seqs)
    maskf = np.asarray(mask).astype(F32)
    trans_np = np.asarray(transitions, dtype=F32)

    nc = _get_compiled()

    in_maps, tstars = [], []
    for c in range(NCORES):
        sl = slice(c * BC, (c + 1) * BC)
        m, ts = _prep_core(feats[sl], tags[sl], maskf[sl], trans_np)
        in_maps.append(m)
        tstars.append(ts)

    res = bass_utils.run_bass_kernel_spmd(nc, in_maps,
                                          core_ids=list(range(NCORES)))

    per_seq = [_stitch_core(res.results[c], tstars[c]) for c in range(NCORES)]
    loss = -np.mean(np.concatenate(per_seq))
    return np.float32(loss)


# revision 12
# speedup vs baseline: 1.6328x; 1.6272x over previous
"""CRF NLL loss kernel for Trainium2 (8 NeuronCores, batch-sharded).

Strategy (v2: time-segmented, rank-1 stitched)
----------------------------------------------
Data-parallel over batch: each core handles BC=64 sequences.  The T=512
forward recurrence (exp domain, labels on partitions, batch on free dim)

    w_t = e_t * (E'^T w_{t-1}),   E' = exp(transitions - C0)

is split into S=16 time segments of LEN=32 steps.  Products of these
strictly positive matrices contract to rank-1 within a few steps, so each
interior segment operator P_s is summarized by its action on probes:
  fwd chains  a_s = P_s r            (seg 0 uses the true init)
  bwd chains  X_s = q^T P_s          merged with EOS-capture injection:
              columns whose sequence ends (t* = len-1) inside seg s carry
              init/injection texp = exp(trans[:,EOS]) at step t* instead
              of the probe; host selects per column.
All 30 chains advance in lockstep rounds r=0..31; per round the DVE does
two joint [128, 960] PSUM->SBUF multiply ops (one per direction), the PE
does 30 matmuls (shared stationaries E', E'^T) plus rank-1 injection
accumulates (stationary texp row).  Features stream in a host-permuted
"need order" (round-major) so DMA/ACT-exp stay ahead of compute with an
8-slot ring; every step's e_t appears twice (once fwd, once bwd).

Host post-processing stitches segments per column with rank-1 junctions
(all dots in float64) and restores the t*.C0 shift; gold path scores are
index-gathered on host and summed on device, as before.
"""

import numpy as np

B, T, L = 512, 512, 128
NCORES = 8
BC = B // NCORES            # 64 sequences per core
PAD, BOS, EOS = 0, 1, 2
C0 = 5.0
S = 16                      # time segments
LEN = 32                    # steps per segment (seg 0: steps 1..32, ...,
                            # seg 15: steps 481..511 -> 31 rounds)
NF = 15                     # fwd chains  (segments 0..14)
NB = 15                     # bwd chains  (segments 1..15)
NSL = NF + NB               # feature slices per round group
NGRP = 33                   # stream groups: [t0] + 32 rounds
RING = 8                    # DMA/exp ring depth (groups)
TMIN = 255                  # smallest possible t* (len >= 256)

F32 = np.float32

_compiled = None


def _t_end(s):
    return 32 * (s + 1) if s <= 14 else 511


def _t_bwd(s, r):
    return 32 * (s + 1) - r if s <= 14 else 511 - r


def _build():
    import concourse.bass as bass
    import concourse.bacc as bacc
    import concourse.mybir as mybir
    import concourse.tile as tile

    f32 = mybir.dt.float32
    bf16 = mybir.dt.bfloat16
    i32 = mybir.dt.int32
    nc = bacc.Bacc("TRN2", target_bir_lowering=False, debug=False)

    featp = nc.dram_tensor("featp", [L, (1 + 32 * NSL) * BC], bf16,
                           kind="ExternalInput")
    trans = nc.dram_tensor("trans", [L, L], f32, kind="ExternalInput")
    trans_t = nc.dram_tensor("trans_t", [L, L], f32, kind="ExternalInput")
    qmask_d = nc.dram_tensor("qmask", [1, NB * BC], bf16, kind="ExternalInput")
    indend_d = nc.dram_tensor("indend", [1, NB * BC], bf16, kind="ExternalInput")
    emis_v = nc.dram_tensor("emis_v", [BC, T], f32, kind="ExternalInput")
    emis_w = nc.dram_tensor("emis_w", [BC, T], f32, kind="ExternalInput")
    trans_v = nc.dram_tensor("trans_v", [BC, T + 1], f32, kind="ExternalInput")
    trans_w = nc.dram_tensor("trans_w", [BC, T + 1], f32, kind="ExternalInput")

    aout = nc.dram_tensor("aout", [L, NF * BC], f32, kind="ExternalOutput")
    xout = nc.dram_tensor("xout", [L, NB * BC], f32, kind="ExternalOutput")
    gold_o = nc.dram_tensor("gold", [BC, 1], f32, kind="ExternalOutput")

    AX = mybir.AxisListType.X
    MUL = mybir.AluOpType.mult
    ADD = mybir.AluOpType.add
    EXP = mybir.ActivationFunctionType.Exp

    GW = NSL * BC               # 1920: cols per full ring group
    FW = NF * BC                # 960

    with tile.TileContext(nc) as tc:
        with (
            tc.tile_pool(name="st", bufs=1) as st,
            tc.tile_pool(name="raws", bufs=RING) as rawp,
            tc.tile_pool(name="efs", bufs=RING) as efp,
            tc.tile_pool(name="pfp", bufs=1, space="PSUM") as pfp,
            tc.tile_pool(name="pbp", bufs=1, space="PSUM") as pbp,
        ):
            # ---- one-time setup ----
            tr_sb = st.tile([L, L], f32)
            nc.sync.dma_start(tr_sb[:], trans[:])
            trt_sb = st.tile([L, L], f32)
            nc.sync.dma_start(trt_sb[:], trans_t[:])
            nc0 = st.tile([L, 1], f32)          # bias tiles
            nc.vector.memset(nc0[:], -C0)
            zb = st.tile([L, 1], f32)
            nc.vector.memset(zb[:], 0.0)
            zb1 = st.tile([1, 1], f32)
            nc.vector.memset(zb1[:], 0.0)
            Ep = st.tile([L, L], bf16)          # E' = exp(trans - C0)
            nc.scalar.activation(Ep[:], tr_sb[:], EXP, bias=nc0[:], scale=1.0)
            EpT = st.tile([L, L], bf16)         # E'^T
            nc.scalar.activation(EpT[:], trt_sb[:], EXP, bias=nc0[:],
                                 scale=1.0)
            texp_f = st.tile([1, L], f32)
            nc.sync.dma_start(texp_f[:],
                              trans[:, EOS:EOS + 1].rearrange("a b -> b a"))
            texp_row = st.tile([1, L], bf16)    # exp(trans[:, EOS]) as a row
            nc.scalar.activation(texp_row[:], texp_f[:], EXP,
                                 bias=zb1[:], scale=1.0)
            ebos = st.tile([L, 1], f32)         # exp(trans[BOS, :]) column
            nc.sync.dma_start(ebos[:],
                              trans[BOS:BOS + 1, :].rearrange("a b -> b a"))
            nc.scalar.activation(ebos[:], ebos[:], EXP, bias=zb[:], scale=1.0)
            # modified bwd stationary: row EOS := texp (injection),
            # col PAD := e_PAD (sustained 1-loop), col EOS := e_PAD.
            nc.sync.dma_start(EpT[EOS:EOS + 1, :], texp_row[:])
            nc.vector.memset(EpT[:, PAD:PAD + 1], 0.0)
            nc.vector.memset(EpT[PAD:PAD + 1, PAD:PAD + 1], 1.0)
            nc.vector.memset(EpT[:, EOS:EOS + 1], 0.0)
            nc.vector.memset(EpT[PAD:PAD + 1, EOS:EOS + 1], 1.0)
            # bwd-init lhsT rows: u1 = ones minus PAD/EOS, u2 = e_PAD + e_EOS
            u1_row = st.tile([1, L], bf16)
            nc.vector.memset(u1_row[:], 1.0)
            nc.vector.memset(u1_row[0:1, PAD:PAD + 1], 0.0)
            nc.vector.memset(u1_row[0:1, EOS:EOS + 1], 0.0)
            u2_row = st.tile([1, L], bf16)
            nc.vector.memset(u2_row[:], 0.0)
            nc.vector.memset(u2_row[0:1, PAD:PAD + 1], 1.0)
            nc.vector.memset(u2_row[0:1, EOS:EOS + 1], 1.0)
            onesb = st.tile([1, BC], bf16)
            nc.vector.memset(onesb[:], 1.0)
            qmask_bf = st.tile([1, NB * BC], bf16)
            nc.sync.dma_start(qmask_bf[:], qmask_d[:])
            indend_bf = st.tile([1, NB * BC], bf16)
            nc.sync.dma_start(indend_bf[:], indend_d[:])

            wring = st.tile([L, FW], bf16)      # fwd states, chain-major
            dring = st.tile([L, FW], bf16)      # bwd delta tiles
            pf = pfp.tile([L, FW], f32, space="PSUM")
            pb = pbp.tile([L, FW], f32, space="PSUM")

            # ---- feature stream: group 0 = t0 slice, groups 1+g = round g
            raw_tiles, ef_tiles = [], []

            def pump(g):
                # DMA + exp one stream group into the rings
                if g == 0:
                    cols = BC
                    base = 0
                else:
                    cols = GW
                    base = BC + (g - 1) * GW
                rt = rawp.tile([L, GW], bf16, tag="raw", name=f"raw{g}")
                nc.sync.dma_start(rt[:, 0:cols], featp[:, base:base + cols])
                # Schraudolph fast-exp on the (otherwise idle) GpSimd:
                # ef = bitcast_f32(int32(x * 2^23/ln2 + B)) ~ exp(x) +-3%
                et = efp.tile([L, GW], i32, tag="ef", name=f"ef{g}")
                nc.gpsimd.tensor_scalar(out=et[:, 0:cols], in0=rt[:, 0:cols],
                                        scalar1=12102203.0,
                                        scalar2=1064866805.0,
                                        op0=MUL, op1=ADD)
                raw_tiles.append(rt)
                ef_tiles.append(et.bitcast(f32))

            for g in range(RING):
                pump(g)

            # ---- init states ----
            # w0 = ebos * e_0 for the true chain; probes = 1
            nc.vector.tensor_scalar(out=wring[:, 0:BC],
                                    in0=ef_tiles[0][:, 0:BC],
                                    scalar1=ebos[:, 0:1], scalar2=None,
                                    op0=MUL)
            nc.vector.memset(wring[:, BC:FW], 1.0)

            # bwd inits into pb: u1 (x) qmask + u2 (x) ones + texp (x) ind_end
            for s in range(1, S):
                sl = pb[:, (s - 1) * BC:s * BC]
                nc.tensor.matmul(sl, lhsT=u1_row[:],
                                 rhs=qmask_bf[0:1, (s - 1) * BC:s * BC],
                                 start=True, stop=False, skip_group_check=True)
                nc.tensor.matmul(sl, lhsT=u2_row[:], rhs=onesb[:],
                                 start=False, stop=False, skip_group_check=True)
                nc.tensor.matmul(sl, lhsT=texp_row[:],
                                 rhs=indend_bf[0:1, (s - 1) * BC:s * BC],
                                 start=False, stop=True, skip_group_check=True)

            # ---- main rounds ----
            # per round r: bwd_DVE(r) [delta = ef_r * pb(r-1)], fwd MMs(r),
            # bwd MMs(r) [consume delta(r), + rank-1 injection], fwd_DVE(r).
            # DVE alternates bwd/fwd joints while PE alternates fwd/bwd MMs.
            for r in range(LEN):
                et = ef_tiles[r + 1]
                nbw = NB if r < LEN - 1 else NB - 1   # chain 15 idle at r=31

                # bwd joint DVE: delta = ef * pb  (pb = last round's outputs)
                nc.vector.tensor_tensor(out=dring[:, 0:nbw * BC],
                                        in0=pb[:, 0:nbw * BC],
                                        in1=et[:, FW:FW + nbw * BC], op=MUL)

                # fwd matmuls (stationary Ep), all 15 chains
                for k in range(NF):
                    nc.tensor.matmul(pf[:, k * BC:(k + 1) * BC], lhsT=Ep[:],
                                     rhs=wring[:, k * BC:(k + 1) * BC],
                                     start=True, stop=True)

                # bwd main matmuls (modified stationary EpT carries the
                # injection: row EOS = texp reads ind from the doctored
                # EOS feature row; col PAD sustains the 1-loop).
                for k in range(nbw):
                    nc.tensor.matmul(pb[:, k * BC:(k + 1) * BC], lhsT=EpT[:],
                                     rhs=dring[:, k * BC:(k + 1) * BC],
                                     start=True, stop=True)

                # fwd joint DVE: w = ef * pf
                nc.vector.tensor_tensor(out=wring[:], in0=pf[:],
                                        in1=et[:, 0:FW], op=MUL)

                # keep the feature ring pumped
                g = r + RING
                if g < NGRP:
                    pump(g)

            # ---- outputs ----
            ast = st.tile([L, FW], f32)
            nc.vector.tensor_copy(ast[:], wring[:])
            nc.sync.dma_start(aout[:], ast[:])
            xst = st.tile([L, FW], f32)
            nc.vector.tensor_copy(xst[:], pb[:])
            nc.sync.dma_start(xout[:], xst[:])

            # ---- gold score masked sums ----
            ev_sb = st.tile([BC, T], f32)
            nc.sync.dma_start(ev_sb[:], emis_v[:])
            ew_sb = st.tile([BC, T], f32)
            nc.sync.dma_start(ew_sb[:], emis_w[:])
            nc.vector.tensor_tensor(out=ev_sb[:], in0=ev_sb[:], in1=ew_sb[:],
                                    op=MUL)
            g1 = st.tile([BC, 1], f32)
            nc.vector.reduce_sum(g1[:], ev_sb[:], axis=AX)
            tv_sb = st.tile([BC, T + 1], f32)
            nc.sync.dma_start(tv_sb[:], trans_v[:])
            tw_sb = st.tile([BC, T + 1], f32)
            nc.sync.dma_start(tw_sb[:], trans_w[:])
            nc.vector.tensor_tensor(out=tv_sb[:], in0=tv_sb[:], in1=tw_sb[:],
                                    op=MUL)
            g2 = st.tile([BC, 1], f32)
            nc.vector.reduce_sum(g2[:], tv_sb[:], axis=AX)
            nc.vector.tensor_tensor(out=g1[:], in0=g1[:], in1=g2[:], op=ADD)
            nc.sync.dma_start(gold_o[:], g1[:])

    nc.compile()
    return nc


def _get_compiled():
    global _compiled
    if _compiled is None:
        _compiled = _build()
    return _compiled


def _bf16(a):
    """fp32 -> bf16 numpy array (ml_dtypes), round-to-nearest-even."""
    import ml_dtypes

    return a.astype(ml_dtypes.bfloat16)


def _stream_perm():
    """Position -> time for the permuted feature stream."""
    pi = np.zeros(1 + 32 * NSL, dtype=np.int64)
    pi[0] = 0
    for r in range(32):
        for k in range(NSL):
            if k < NF:
                t = 32 * k + r + 1
            else:
                t = _t_bwd(k - NF + 1, r)
            pi[1 + r * NSL + k] = t
    return pi


_PI = _stream_perm()


def _prep_core(feat, tags, maskf, trans_np):
    """Host-side marshalling for one core's shard."""
    lens = maskf.sum(axis=1).astype(np.int64)
    tstar = lens - 1                                   # in [255, 511]

    fT = np.ascontiguousarray(feat.transpose(2, 1, 0))  # [L, T, BC]
    featp = np.ascontiguousarray(fT[:, _PI, :])        # [L, 961, BC]
    # doctor the bwd slices: PAD row sustains ~1.0 through the fast-exp
    # (PADVAL maps to float 1.0 bits); EOS row carries the injection
    # indicator for the next processed step (PADVAL -> ~1, NOINJ -> ~0).
    PADVAL = (1065353216.0 - 1064866805.0) / 12102203.0
    NOINJ = -60.0
    for r in range(32):
        for k in range(NF, NSL):
            s = k - NF + 1
            pos = 1 + r * NSL + k
            t = _t_bwd(s, r)
            if s == 15 and r == 31:
                continue                      # unused slot
            featp[PAD, pos, :] = PADVAL
            tn = t - 1
            if TMIN <= tn <= 511 and tn >= 32 * s + 1:
                featp[EOS, pos, :] = np.where(tstar == tn, PADVAL, NOINJ)
            else:
                featp[EOS, pos, :] = NOINJ
    featp = _bf16(featp.reshape(L, -1))

    qmask = np.zeros((1, NB * BC), dtype=F32)
    indend = np.zeros((1, NB * BC), dtype=F32)
    for s in range(1, S):
        te = _t_end(s)
        qmask[0, (s - 1) * BC:s * BC] = (tstar > te)
        indend[0, (s - 1) * BC:s * BC] = (tstar == te)

    emis_v = np.take_along_axis(feat, tags[..., None], axis=-1)[..., 0]
    emis_w = maskf.copy()
    emis_w[:, 0] = 1.0
    trans_v = np.empty((BC, T + 1), dtype=F32)
    trans_v[:, : T - 1] = trans_np[tags[:, :-1], tags[:, 1:]]
    trans_v[:, T - 1] = trans_np[BOS, tags[:, 0]]
    last_lab = tags[np.arange(BC), tstar]
    trans_v[:, T] = trans_np[last_lab, EOS]
    trans_w = np.empty((BC, T + 1), dtype=F32)
    trans_w[:, : T - 1] = maskf[:, 1:]
    trans_w[:, T - 1] = 1.0
    trans_w[:, T] = 1.0

    in_map = {
        "featp": featp,
        "trans": np.ascontiguousarray(trans_np),
        "trans_t": np.ascontiguousarray(trans_np.T),
        "qmask": _bf16(qmask),
        "indend": _bf16(indend),
        "emis_v": np.ascontiguousarray(emis_v.astype(F32)),
        "emis_w": np.ascontiguousarray(emis_w),
        "trans_v": trans_v,
        "trans_w": trans_w,
    }
    return in_map, tstar


def _prep_all(inputs):
    feats = np.asarray(inputs["features"], dtype=F32)
    tags = np.asarray(inputs["tag_seqs"])
    maskf = np.asarray(inputs["mask"]).astype(F32)
    tr = np.asarray(inputs["transitions"], dtype=F32)
    in_maps = []
    for c in range(NCORES):
        sl = slice(c * BC, (c + 1) * BC)
        m, _ = _prep_core(feats[sl], tags[sl], maskf[sl], tr)
        in_maps.append(m)
    return in_maps


def _stitch_core(out, tstar):
    """Host rank-1 stitching for one core -> per-seq (gold - logZ)."""
    a = np.asarray(out["aout"], dtype=np.float64).reshape(L, NF, BC)
    X = np.asarray(out["xout"], dtype=np.float64).reshape(L, NB, BC)
    gold = np.asarray(out["gold"], dtype=np.float64)[:, 0]

    # dots: m1[s] = log(X_{s+1} . a_s)  s=1..14 ; mfin = log(X_1 . a_0)
    # msum[s] = log(sum a_s)
    live = np.ones(L, bool)
    live[[PAD, EOS]] = False
    with np.errstate(divide="ignore"):
        m1 = np.log(np.einsum("lsb,lsb->sb", X[:, 1:, :], a[:, 1:, :]))
        msum = np.log(a[live][:, 1:, :].sum(axis=0))   # [14, BC] s=1..14
        mfin = np.log(np.einsum("lb,lb->b", X[:, 0, :], a[:, 0, :]))

    sstar = (tstar - 1) // 32                          # in [7, 15]
    # logZdev = sum_{s=1}^{sstar-1} (m1[s] - msum[s]) + mfin
    # m1 index: s -> m1[s-1]
    cum = np.concatenate([np.zeros((1, BC)),
                          np.cumsum(m1 - msum, axis=0)], axis=0)  # [15, BC]
    logZdev = cum[sstar - 1, np.arange(BC)] + mfin
    logZ = logZdev + tstar * C0
    return gold - logZ


def kernel(features, tag_seqs, mask, transitions):
    from concourse import bass_utils

    feats = np.asarray(features, dtype=F32)
    tags = np.asarray(tag_seqs)
    maskf = np.asarray(mask).astype(F32)
    trans_np = np.asarray(transitions, dtype=F32)

    nc = _get_compiled()

    in_maps, tstars = [], []
    for c in range(NCORES):
        sl = slice(c * BC, (c + 1) * BC)
        m, ts = _prep_core(feats[sl], tags[sl], maskf[sl], trans_np)
        in_maps.append(m)
        tstars.append(ts)

    res = bass_utils.run_bass_kernel_spmd(nc, in_maps,
                                          core_ids=list(range(NCORES)))

    per_seq = [_stitch_core(res.results[c], tstars[c]) for c in range(NCORES)]
    loss = -np.mean(np.concatenate(per_seq))
    return np.float32(loss)
